# revision 66
# baseline (speedup 1.0000x reference)
"""Trainium2 Bass kernel for nn_BERTEmbedding (fused per-index affine + sinusoidal PE).

Math (per batch b, vocab-position v, embed index e):
    out[b,v,e] = s0[b,v]*flux_w[v,e] + flux_b[v,e]
               + s2[b,v]*time_w[v,e] + time_b[v,e]
               + (e even: sin(s1[b,v]*div[e/2]) ; e odd: cos(s1[b,v]*div[(e-1)/2]))

Sharding: vocab axis V=4096 split across 8 cores (512 rows each); every core
handles all 16 batches of its vocab shard.  Weight tables are sharded with the
vocab axis.

Device strategy (per core, 4 v-tiles x 16 batches = 64 work items of [128,768],
software-pipelined in 8 stages of GB=8 batches; 107.0us HW, rel err 4.9e-3):
  - Output stored BF16 (halves the dominant store traffic; host converts back
    to f32), one batched 1.57MB store per stage instead of 8 per-item stores.
  - pe tiles (x3 ping-pong) live in the NATIVE interleaved layout (even col
    2k = sin_k, cos at 2k+1) so the DVE merge out = psum + pe is a flat AP.
  - Small-angle cuts: cos lanes k >= COSH=197 are constant 1.0 (memset once);
    sin lanes k >= KSIN=138 equal ang itself - the ScalarE staging op writes
    ang straight into those pe lanes (stride-2 dst) and Sin only runs on
    [KLO,COSH) cos / [KLO,KSIN) sin heads, in place over the staged angles.
  - TensorE: psum = diag(s0) @ fw + diag(s2) @ tw + I @ bsum, bf16 weights,
    f32 PSUM accumulate, 512/256 column splits per PSUM bank,
    stationary-major matmul order.
  - ScalarE per stage: 8 per-item staging muls (ang = s1 * dv, plain 2D
    scale APs only) + 3 grouped Sins.  ~71us busy.
  - VectorE: 64 direct merges (psum f32 + pe bf16 -> bf16, 1x ~960ns) +
    one grouped lo-lane staging mult per stage.  ~71us busy.
  - GPSIMD: diag builds D[p,b*128+q] = eye[p,q]*s_ch[p,b] (broadcast chunks)
    + the lo-lane r' = r0 + combo grouped add.  k < KLO lanes use host phase
    codes combo_n and Sin(scale=pi/2) as before.
  - DMA: all loads + stores on the SP (sync) HWDGE queue.
NB (measured, load-bearing):
  - GPSIMD >~60us busy starves concurrent DVE ops via the shared SBUF port
    (gpsimd.tensor_scalar is ~6us for [128,432] - never use it).
  - TRN2 matmul output must be f32 (bf16 PSUM is TRN3-only), so the merge is
    pinned at 1x; ScalarE-evacuation + 2x bf16 adds measured ~540+865ns but
    the extra ACT hop serializes the pipeline - direct merges win.
  - Per-item DVE tensor_scalar with an AP scalar runs ~700ns for 336 lanes
    (no 2x mode); grouped broadcast TT is ~1.2ns/elem but loads the DVE.
  - Engine busys at 107us: DVE 71, ACT 71, PE 72 (3-way balanced), GPSIMD 47,
    DMA 54.  PE floor is 3 matmul terms x 49152 cols / 2.4GHz = 61us.
"""

import math

import numpy as np

try:
    import concourse.bass as bass
except ImportError:  # harness containers keep the repo at /opt/trn_rl_repo
    import sys

    sys.path.insert(0, "/opt/trn_rl_repo")
    import concourse.bass as bass

import concourse.bacc as bacc
import concourse.tile as tile
from concourse import mybir
from concourse.bass_utils import run_bass_kernel_spmd

B, V, E = 16, 4096, 768
EH = E // 2  # 384 angle lanes
KLO = 48  # angle lanes fixed up via the host combo tensor
KHI = EH - KLO  # 336 direct-sin lanes
# cos(ang_k) = 1.0 within ~1e-3 for k >= COSH (|ang| <= S1_LIMIT*10^(-k/96));
# those pe lanes are memset once instead of computed by ScalarE
COSH = 197
# sin(ang_k) = ang_k within ~1e-3 for k >= KSIN; staging writes ang into the
# pe tile directly and ScalarE only evaluates Sin on lanes [KLO, KSIN)
KSIN = 138
N_CORES = 8
V_SHARD = V // N_CORES  # 512
# merges per stage read psum directly on the DVE (1x); the rest go through a
# ScalarE psum->bf16 evacuation and an all-bf16 2x DVE add
N_DIRECT = 8
VT = V_SHARD // 128  # 4 v-tiles per core
GB = 8  # batches per pe/sin group
F32 = mybir.dt.float32
BF16 = mybir.dt.bfloat16

TWO_PI = 2.0 * math.pi
HALF_PI = float(np.float32(math.pi / 2.0))
# keep reduced angles strictly inside ScalarE's [-pi, pi] spline domain
SIN_SAFETY = 1.0 - 1e-6
# direct-Sin lanes need |s1|*d_KLO + pi/2 <= pi
S1_LIMIT = (math.pi / 2.0) / math.exp(-KLO * math.log(10000.0) / EH)



def build_bass() -> "bass.Bass":
    from contextlib import ExitStack

    nc = bacc.Bacc(
        "TRN2",
        target_bir_lowering=False,
        debug=False,
        num_devices=N_CORES,
    )
    Alu = mybir.AluOpType

    # dv_full = [ div[KLO:EH) (336) | dv2' (96, scaled 2/pi) ]
    KX = KHI + 2 * KLO  # 432
    seq_d = nc.dram_tensor("seq", [128, VT * B * 3], F32, kind="ExternalInput")
    fw_d = nc.dram_tensor("fw", [V_SHARD, E], BF16, kind="ExternalInput")
    tw_d = nc.dram_tensor("tw", [V_SHARD, E], BF16, kind="ExternalInput")
    bs_d = nc.dram_tensor("bs", [V_SHARD, E], BF16, kind="ExternalInput")
    dv_d = nc.dram_tensor("dv", [128, KX], F32, kind="ExternalInput")
    cmb_d = nc.dram_tensor("combo", [128, VT * B * 2 * KLO], BF16, kind="ExternalInput")
    eye_d = nc.dram_tensor("eye", [128, 128], BF16, kind="ExternalInput")
    # bf16 output, one [128, E] block per work item (vt*B + b); host reassembles
    out_d = nc.dram_tensor("out", [VT * B, 128, E], BF16, kind="ExternalOutput")

    with tile.TileContext(nc) as tc, ExitStack() as ctx:
        const_pool = ctx.enter_context(tc.tile_pool(name="const", bufs=1))
        tab_pool = ctx.enter_context(tc.tile_pool(name="tables", bufs=2))
        dmat_pool = ctx.enter_context(tc.tile_pool(name="dmat", bufs=4))
        ang_pool = ctx.enter_context(tc.tile_pool(name="ang", bufs=4))
        ev_pool = ctx.enter_context(tc.tile_pool(name="ev", bufs=6))
        out_pool = ctx.enter_context(tc.tile_pool(name="out", bufs=4))
        psum_pool = ctx.enter_context(tc.tile_pool(name="psum", bufs=2, space="PSUM"))

        zero_t = const_pool.tile([128, 1], F32, tag="zero")
        nc.vector.memset(zero_t[:], 0.0)
        hpi_t = const_pool.tile([128, 1], F32, tag="hpi")
        nc.vector.memset(hpi_t[:], HALF_PI)
        # dummy Sin up front: forces the ~2.7us ACT_TABLE_LOAD to overlap the
        # initial DMAs instead of stalling the first real Sin mid-pipeline
        warm_t = const_pool.tile([128, 1], F32, tag="warm")
        nc.scalar.activation(
            warm_t[:], zero_t[:], mybir.ActivationFunctionType.Sin,
            bias=zero_t[:], scale=1.0,
        )

        seq_t = const_pool.tile([128, VT * B * 3], F32, tag="seq")
        nc.sync.dma_start(seq_t[:], seq_d[:])
        dv_t = const_pool.tile([128, KX], F32, tag="dv")
        nc.sync.dma_start(dv_t[:], dv_d[:])
        eye_t = const_pool.tile([128, 128], BF16, tag="eye")
        nc.sync.dma_start(eye_t[:], eye_d[:])

        # ping-pong pe tiles in the NATIVE interleaved layout (even col 2k =
        # sin lane k, odd col 2k+1 = cos lane k) so merges read flat APs.
        # cos lanes k >= COSH hold constant 1.0 (memset once, never
        # overwritten: ScalarE only writes sin + cos-head lanes)
        pe_tiles = []
        for pp in range(3):
            pe_t = const_pool.tile([128, GB * E], BF16, tag=f"pe{pp}")
            nc.vector.memset(
                pe_t[:].rearrange("p (i e) -> p i e", i=GB)[
                    :, :, 2 * COSH + 1 : E : 2
                ],
                1.0,
            )
            pe_tiles.append(pe_t)

        # ---- software-pipelined emission over uneven stages (vt, b0, nb) ----
        # Stage k's production (angles, r4, sins, next-vtile diag builds) is
        # emitted BEFORE stage k-1's consumption (matmuls, merges, stores) so
        # no engine queue head-of-line blocks on a consumer stall.  The first
        # and last stages are half-size so the pipeline fill and drain ramps
        # are governed by a 4-item chain instead of an 8-item one.
        stages = [(0, 0, 2), (0, 2, 2), (0, 4, 4), (0, 8, 8)]
        for vt in range(1, VT - 1):
            stages += [(vt, 0, 8), (vt, 8, 8)]
        stages += [(VT - 1, 0, 8), (VT - 1, 8, 4), (VT - 1, 12, 2), (VT - 1, 14, 2)]
        tabs: dict = {}
        dmats: dict = {}
        staged: dict = {}

        def seq_view(vt):
            return seq_t[:, vt * B * 3 : (vt + 1) * B * 3].rearrange(
                "p (b c) -> p b c", c=3
            )

        def load_tables(vt):
            fw_t = tab_pool.tile([128, E], BF16, tag="fw")
            nc.sync.dma_start(fw_t[:], fw_d[vt * 128 : (vt + 1) * 128, :])
            tw_t = tab_pool.tile([128, E], BF16, tag="tw")
            nc.sync.dma_start(tw_t[:], tw_d[vt * 128 : (vt + 1) * 128, :])
            bs_t = tab_pool.tile([128, E], BF16, tag="bs")
            nc.sync.dma_start(bs_t[:], bs_d[vt * 128 : (vt + 1) * 128, :])
            cmb_t = tab_pool.tile([128, B * 2 * KLO], BF16, tag="cmb")
            nc.sync.dma_start(
                cmb_t[:], cmb_d[:, vt * B * 2 * KLO : (vt + 1) * B * 2 * KLO]
            )
            tabs[vt] = (fw_t, tw_t, bs_t, cmb_t)

        def alloc_dmats(vt):
            dmats[vt] = {}
            for ch in (0, 2):
                d_t = dmat_pool.tile([128, B * 128], BF16, tag=f"d{ch}")
                dmats[vt][ch] = d_t

        def build_d(vt):
            # diag build D[p, b*128+q] = eye[p,q] * s_ch[p,b]:
            # batched broadcast chunks on GPSIMD for every v-tile
            sv = seq_view(vt)
            # first-half chunks (batches 0..7) for BOTH channels first: that
            # is all the g=0 stage needs, so its matmuls start ~4us earlier
            for h0 in (0, B // 2):
                for ch in (0, 2):
                    d3 = dmats[vt][ch][:].rearrange("p (b q) -> p b q", b=B)
                    eye_b = eye_t[:].unsqueeze(1).broadcast_to((128, B // 2, 128))
                    s_b = sv[:, h0 : h0 + B // 2, ch : ch + 1].broadcast_to(
                        (128, B // 2, 128)
                    )
                    nc.gpsimd.tensor_tensor(
                        d3[:, h0 : h0 + B // 2, :], eye_b, s_b, Alu.mult
                    )

        def emit_stage(k, vt, b0, nb):
            ang_g = ang_pool.tile([128, GB * 2 * KLO], F32, tag="ang")
            r4_g = ang_pool.tile([128, GB * 2 * KLO], F32, tag="r4")
            cmb_t = tabs[vt][3]
            pe_g = pe_tiles[k % 3]
            pe3 = pe_g[:].rearrange("p (i e) -> p i e", i=GB)[:, 0:nb, :]
            sv = seq_view(vt)
            s1b = sv[:, b0 : b0 + nb, 1:2]  # [128, nb, 1] positional channel
            # staging op2': ang_k for k >= KLO straight into the pe sin
            # lanes (sin(x)=x holds for k >= KSIN; lanes [KLO,KSIN) are
            # Sin-ed in place below; lanes [KSIN,COSH) feed cos first).
            # Per-item on ScalarE: the DVE queue (merges) is the pacer.
            for i, b in enumerate(range(b0, b0 + nb)):
                col = vt * B * 3 + b * 3 + 1
                nc.scalar.mul(
                    pe_g[:, i * E + 2 * KLO : (i + 1) * E : 2],
                    dv_t[:, 0:KHI],
                    seq_t[:, col : col + 1],
                )
            # grouped staging op1': lo r0 lanes (dv2' scaled 2/pi) — on the
            # DVE: GPSIMD runs these ~3x slower and its SBUF-port traffic
            # stalls concurrent DVE work
            nc.vector.tensor_tensor(
                ang_g[:, 0 : nb * 2 * KLO].rearrange("p (i l) -> p i l", i=nb),
                dv_t[:, KHI:KX].unsqueeze(1).broadcast_to((128, nb, 2 * KLO)),
                s1b.broadcast_to((128, nb, 2 * KLO)),
                Alu.mult,
            )
            # lo codes r' = r0 + combo, one grouped add
            nc.gpsimd.tensor_tensor(
                r4_g[:, 0 : nb * 2 * KLO].rearrange("p (i l) -> p i l", i=nb),
                ang_g[:, 0 : nb * 2 * KLO].rearrange("p (i l) -> p i l", i=nb),
                cmb_t[
                    :, b0 * 2 * KLO : (b0 + nb) * 2 * KLO
                ].rearrange("p (i l) -> p i l", i=nb),
                Alu.add,
            )
            # cos head: reads the staged angles from the sin lanes (before
            # the in-place Sin consumes them), writes the odd cos columns
            nc.scalar.activation(
                pe3[:, :, 2 * KLO + 1 : 2 * COSH : 2],
                pe3[:, :, 2 * KLO : 2 * COSH : 2],
                mybir.ActivationFunctionType.Sin,
                bias=hpi_t[:],
                scale=1.0,
            )
            # sin head, in place over the staged angles
            nc.scalar.activation(
                pe3[:, :, 2 * KLO : 2 * KSIN : 2],
                pe3[:, :, 2 * KLO : 2 * KSIN : 2],
                mybir.ActivationFunctionType.Sin,
                bias=zero_t[:],
                scale=1.0,
            )
            # lo block: r4 layout per item is [48 sin | 48 cos]
            nc.scalar.activation(
                pe3[:, :, 0 : 2 * KLO].rearrange("p i (q h) -> p i h q", h=2),
                r4_g[:, 0 : nb * 2 * KLO].rearrange(
                    "p (i h q) -> p i h q", i=nb, h=2
                ),
                mybir.ActivationFunctionType.Sin,
                bias=zero_t[:],
                scale=HALF_PI * SIN_SAFETY,
            )
            return pe_g

        def emit_items(vt, b0, nb, pe_g):
            fw_t, tw_t, bs_t, _ = tabs[vt]
            # bf16 output written per merge-pair straight to item-keyed DRAM
            # blocks: uniform 393KB stores keep the tail drain overlapped
            o_g = out_pool.tile([128, GB * E], BF16, tag="o")
            for i, b in enumerate(range(b0, b0 + nb)):
                # items are merged in PAIRS from one 4-bank psum tile
                # (item0 at cols [0:768), item1 at [1024:1792)): one DVE
                # tensor_tensor per pair amortizes the 120-cycle PSUM-access
                # overhead and halves the DVE op count
                if i % 2 == 0:
                    psp = psum_pool.tile([128, 2048], F32, tag="ps")
                base = (i % 2) * 1024
                # psum += diag(s0)@fw + diag(s2)@tw + I@bsum, 512/256 column
                # splits per PSUM bank, stationary-major order
                for st_t, mov_t, first, last in (
                    (dmats[vt][0][:, b * 128 : (b + 1) * 128], fw_t, True, False),
                    (dmats[vt][2][:, b * 128 : (b + 1) * 128], tw_t, False, False),
                    (eye_t[:], bs_t, False, True),
                ):
                    for lo, hi in ((0, 512), (512, E)):
                        nc.tensor.matmul(
                            psp[:, base + lo : base + hi],
                            st_t,
                            mov_t[:, lo:hi],
                            start=first,
                            stop=last,
                        )
                if i % 2 == 1:
                    # merge both items of the pair (flat APs: pe is natively
                    # interleaved; psum f32 caps the TT at 1x)
                    nc.vector.tensor_tensor(
                        o_g[:, (i - 1) * E : (i + 1) * E].rearrange(
                            "p (j e) -> p j e", j=2
                        ),
                        psp[:].rearrange("p (j x) -> p j x", j=2)[:, :, 0:E],
                        pe_g[:, (i - 1) * E : (i + 1) * E].rearrange(
                            "p (j e) -> p j e", j=2
                        ),
                        Alu.add,
                    )
                if i % 2 == 1:
                    it = vt * B + b  # global item index of the pair's 2nd item
                    nc.sync.dma_start(
                        out_d[it - 1 : it + 1].rearrange("j p e -> p j e"),
                        o_g[:, (i - 1) * E : (i + 1) * E].rearrange(
                            "p (j e) -> p j e", j=2
                        ),
                    )

        load_tables(0)
        alloc_dmats(0)
        build_d(0)
        for k, (vt, b0, nb) in enumerate(stages):
            if b0 == 0 and vt + 1 < VT:
                load_tables(vt + 1)
            staged[(vt, b0)] = emit_stage(k, vt, b0, nb)
            if b0 == 0 and vt + 1 < VT:
                alloc_dmats(vt + 1)
                build_d(vt + 1)
            if k >= 1:
                pvt, pb0, pnb = stages[k - 1]
                emit_items(pvt, pb0, pnb, staged.pop((pvt, pb0)))
        pvt, pb0, pnb = stages[-1]
        emit_items(pvt, pb0, pnb, staged.pop((pvt, pb0)))


    nc.finalize()
    return nc


_NC_CACHE: list = []


def _get_nc():
    if not _NC_CACHE:
        _NC_CACHE.append(build_bass())
    return _NC_CACHE[0]


def make_in_maps(sequence, flux_w, flux_b, time_w, time_b):
    import ml_dtypes

    sequence = np.asarray(sequence, dtype=np.float32)
    flux_w = np.asarray(flux_w, dtype=np.float32)
    time_w = np.asarray(time_w, dtype=np.float32)
    bsum = np.asarray(flux_b, dtype=np.float32) + np.asarray(time_b, dtype=np.float32)

    s1_all = sequence[:, :, 1]
    assert np.abs(s1_all).max() < S1_LIMIT, (
        f"positional channel exceeds direct-Sin range: {np.abs(s1_all).max():.3f} "
        f">= {S1_LIMIT:.3f}; raise KLO"
    )

    div = np.exp(
        np.arange(0, E, 2, dtype=np.float32) * np.float32(-math.log(10000.0) / E)
    ).astype(np.float32)
    # dv_full = [ div[KLO:] (336) | 48 lo sin lanes * 2/pi | 48 lo cos * 2/pi ]
    dv2p = (np.concatenate([div[:KLO], div[:KLO]]) * np.float32(2.0 / math.pi)).astype(
        np.float32
    )
    dv_ext = np.concatenate([div[KLO:], dv2p]).astype(np.float32)
    dv_rep = np.ascontiguousarray(np.broadcast_to(dv_ext, (128, KHI + 2 * KLO)))
    eye_bf = np.eye(128, dtype=np.float32).astype(ml_dtypes.bfloat16)

    # combo_n[b,v,h*KLO+k] = j - 4*rint((s1*d_k + j*pi/2)/2pi), j = h (0=sin,1=cos)
    jj = np.concatenate([np.zeros(KLO, np.float64), np.ones(KLO, np.float64)])
    dd = np.concatenate([div[:KLO], div[:KLO]]).astype(np.float64)
    ang = s1_all[:, :, None].astype(np.float64) * dd[None, None, :] + jj * (
        math.pi / 2.0
    )
    n = np.rint(ang / TWO_PI)
    combo_n = (jj[None, None, :] - 4.0 * n).astype(np.float32)
    assert np.abs(combo_n).max() <= 16, "combo codes exceed bf16-exact range"
    combo_bf = combo_n.astype(ml_dtypes.bfloat16)  # small ints: bf16-exact

    fw_bf = flux_w.astype(ml_dtypes.bfloat16)
    tw_bf = time_w.astype(ml_dtypes.bfloat16)
    bs_bf = bsum.astype(ml_dtypes.bfloat16)

    in_maps = []
    for c in range(N_CORES):
        v0, v1 = c * V_SHARD, (c + 1) * V_SHARD
        # [B, 512, 3] -> [128p, vt*B*3 + b*3 + ch]
        s = sequence[:, v0:v1, :].reshape(B, VT, 128, 3)
        seq_r = np.ascontiguousarray(s.transpose(2, 1, 0, 3)).reshape(128, VT * B * 3)
        # combo [B, 512, 2*KLO] -> [128p, (vt*B + b)*2*KLO + lane]
        cmb = combo_bf[:, v0:v1, :].reshape(B, VT, 128, 2 * KLO)
        cmb_r = np.ascontiguousarray(cmb.transpose(2, 1, 0, 3)).reshape(
            128, VT * B * 2 * KLO
        )
        in_maps.append(
            {
                "seq": seq_r,
                "fw": np.ascontiguousarray(fw_bf[v0:v1]),
                "tw": np.ascontiguousarray(tw_bf[v0:v1]),
                "bs": np.ascontiguousarray(bs_bf[v0:v1]),
                "dv": dv_rep,
                "combo": cmb_r,
                "eye": eye_bf,
            }
        )
    return in_maps


def run(in_maps, trace: bool = False):
    nc = _get_nc()
    return run_bass_kernel_spmd(nc, in_maps, list(range(N_CORES)), trace=trace)


def assemble(res) -> np.ndarray:
    """Reassemble per-core [VT*B, 128, E] bf16 item blocks into [B, V, E] f32."""
    cores = []
    for c in range(N_CORES):
        arr = np.asarray(res.results[c]["out"]).astype(np.float32)
        # [vt*B+b, p, e] -> [b, v, e] with v = vt*128+p
        arr = arr.reshape(VT, B, 128, E).transpose(1, 0, 2, 3)
        cores.append(arr.reshape(B, V_SHARD, E))
    return np.ascontiguousarray(np.concatenate(cores, axis=1))


def kernel(sequence, flux_w, flux_b, time_w, time_b) -> np.ndarray:
    in_maps = make_in_maps(sequence, flux_w, flux_b, time_w, time_b)
    res = run(in_maps)
    return assemble(res)



# revision 67
# speedup vs baseline: 1.0817x; 1.0817x over previous
"""Trainium2 Bass kernel for nn_BERTEmbedding (fused per-index affine + sinusoidal PE).

Math (per batch b, vocab-position v, embed index e):
    out[b,v,e] = s0[b,v]*flux_w[v,e] + flux_b[v,e]
               + s2[b,v]*time_w[v,e] + time_b[v,e]
               + (e even: sin(s1[b,v]*div[e/2]) ; e odd: cos(s1[b,v]*div[(e-1)/2]))

Sharding: vocab axis V=4096 split across 8 cores (512 rows each); every core
handles all 16 batches of its vocab shard.  Weight tables are sharded with the
vocab axis.

Device strategy (per core, 4 v-tiles x 16 batches = 64 work items of [128,768],
software-pipelined in 8 stages of GB=8 batches; 107.0us HW, rel err 4.9e-3):
  - Output stored BF16 (halves the dominant store traffic; host converts back
    to f32), one batched 1.57MB store per stage instead of 8 per-item stores.
  - pe tiles (x3 ping-pong) live in the NATIVE interleaved layout (even col
    2k = sin_k, cos at 2k+1) so the DVE merge out = psum + pe is a flat AP.
  - Small-angle cuts: cos lanes k >= COSH=197 are constant 1.0 (memset once);
    sin lanes k >= KSIN=138 equal ang itself - the ScalarE staging op writes
    ang straight into those pe lanes (stride-2 dst) and Sin only runs on
    [KLO,COSH) cos / [KLO,KSIN) sin heads, in place over the staged angles.
  - TensorE: psum = diag(s0) @ fw + diag(s2) @ tw + I @ bsum, bf16 weights,
    f32 PSUM accumulate, 512/256 column splits per PSUM bank,
    stationary-major matmul order.
  - ScalarE per stage: 8 per-item staging muls (ang = s1 * dv, plain 2D
    scale APs only) + 3 grouped Sins.  ~71us busy.
  - VectorE: 64 direct merges (psum f32 + pe bf16 -> bf16, 1x ~960ns) +
    one grouped lo-lane staging mult per stage.  ~71us busy.
  - GPSIMD: diag builds D[p,b*128+q] = eye[p,q]*s_ch[p,b] (broadcast chunks)
    + the lo-lane r' = r0 + combo grouped add.  k < KLO lanes use host phase
    codes combo_n and Sin(scale=pi/2) as before.
  - DMA: all loads + stores on the SP (sync) HWDGE queue.
NB (measured, load-bearing):
  - GPSIMD >~60us busy starves concurrent DVE ops via the shared SBUF port
    (gpsimd.tensor_scalar is ~6us for [128,432] - never use it).
  - TRN2 matmul output must be f32 (bf16 PSUM is TRN3-only), so the merge is
    pinned at 1x; ScalarE-evacuation + 2x bf16 adds measured ~540+865ns but
    the extra ACT hop serializes the pipeline - direct merges win.
  - Per-item DVE tensor_scalar with an AP scalar runs ~700ns for 336 lanes
    (no 2x mode); grouped broadcast TT is ~1.2ns/elem but loads the DVE.
  - Engine busys at 107us: DVE 71, ACT 71, PE 72 (3-way balanced), GPSIMD 47,
    DMA 54.  PE floor is 3 matmul terms x 49152 cols / 2.4GHz = 61us.
"""

import math

import numpy as np

try:
    import concourse.bass as bass
except ImportError:  # harness containers keep the repo at /opt/trn_rl_repo
    import sys

    sys.path.insert(0, "/opt/trn_rl_repo")
    import concourse.bass as bass

import concourse.bacc as bacc
import concourse.tile as tile
from concourse import mybir
from concourse.bass_utils import run_bass_kernel_spmd

B, V, E = 16, 4096, 768
EH = E // 2  # 384 angle lanes
KLO = 48  # angle lanes fixed up via the host combo tensor
KHI = EH - KLO  # 336 direct-sin lanes
# cos(ang_k) = 1.0 within ~1e-3 for k >= COSH (|ang| <= S1_LIMIT*10^(-k/96));
# those pe lanes are memset once instead of computed by ScalarE
COSH = 197
# sin(ang_k) = ang_k within ~1e-3 for k >= KSIN; staging writes ang into the
# pe tile directly and ScalarE only evaluates Sin on lanes [KLO, KSIN)
KSIN = 138
N_CORES = 8
V_SHARD = V // N_CORES  # 512
# merges per stage read psum directly on the DVE (1x); the rest go through a
# ScalarE psum->bf16 evacuation and an all-bf16 2x DVE add
N_DIRECT = 8
VT = V_SHARD // 128  # 4 v-tiles per core
GB = 8  # batches per pe/sin group
F32 = mybir.dt.float32
BF16 = mybir.dt.bfloat16

TWO_PI = 2.0 * math.pi
HALF_PI = float(np.float32(math.pi / 2.0))
# keep reduced angles strictly inside ScalarE's [-pi, pi] spline domain
SIN_SAFETY = 1.0 - 1e-6
# direct-Sin lanes need |s1|*d_KLO + pi/2 <= pi
S1_LIMIT = (math.pi / 2.0) / math.exp(-KLO * math.log(10000.0) / EH)



def build_bass() -> "bass.Bass":
    from contextlib import ExitStack

    nc = bacc.Bacc(
        "TRN2",
        target_bir_lowering=False,
        debug=False,
        num_devices=N_CORES,
    )
    Alu = mybir.AluOpType

    # dv_full = [ div[KLO:EH) (336) | dv2' (96, scaled 2/pi) ]
    KX = KHI + 2 * KLO  # 432
    seq_d = nc.dram_tensor("seq", [128, VT * B * 3], F32, kind="ExternalInput")
    fw_d = nc.dram_tensor("fw", [V_SHARD, E], BF16, kind="ExternalInput")
    tw_d = nc.dram_tensor("tw", [V_SHARD, E], BF16, kind="ExternalInput")
    bs_d = nc.dram_tensor("bs", [V_SHARD, E], BF16, kind="ExternalInput")
    dv_d = nc.dram_tensor("dv", [128, KX], F32, kind="ExternalInput")
    cmb_d = nc.dram_tensor("combo", [128, VT * B * 2 * KLO], BF16, kind="ExternalInput")
    eye_d = nc.dram_tensor("eye", [128, 128], BF16, kind="ExternalInput")
    # bf16 output, one [128, E] block per work item (vt*B + b); host reassembles
    out_d = nc.dram_tensor("out", [VT * B, 128, E], BF16, kind="ExternalOutput")

    with tile.TileContext(nc) as tc, ExitStack() as ctx:
        const_pool = ctx.enter_context(tc.tile_pool(name="const", bufs=1))
        tab_pool = ctx.enter_context(tc.tile_pool(name="tables", bufs=2))
        dmat_pool = ctx.enter_context(tc.tile_pool(name="dmat", bufs=4))
        ang_pool = ctx.enter_context(tc.tile_pool(name="ang", bufs=4))
        ev_pool = ctx.enter_context(tc.tile_pool(name="ev", bufs=6))
        out_pool = ctx.enter_context(tc.tile_pool(name="out", bufs=4))
        psum_pool = ctx.enter_context(tc.tile_pool(name="psum", bufs=2, space="PSUM"))

        zero_t = const_pool.tile([128, 1], F32, tag="zero")
        nc.vector.memset(zero_t[:], 0.0)
        hpi_t = const_pool.tile([128, 1], F32, tag="hpi")
        nc.vector.memset(hpi_t[:], HALF_PI)
        # dummy Sin up front: forces the ~2.7us ACT_TABLE_LOAD to overlap the
        # initial DMAs instead of stalling the first real Sin mid-pipeline
        warm_t = const_pool.tile([128, 1], F32, tag="warm")
        nc.scalar.activation(
            warm_t[:], zero_t[:], mybir.ActivationFunctionType.Sin,
            bias=zero_t[:], scale=1.0,
        )

        seq_t = const_pool.tile([128, VT * B * 3], F32, tag="seq")
        nc.sync.dma_start(seq_t[:], seq_d[:])
        dv_t = const_pool.tile([128, KX], F32, tag="dv")
        nc.sync.dma_start(dv_t[:], dv_d[:])
        eye_t = const_pool.tile([128, 128], BF16, tag="eye")
        nc.sync.dma_start(eye_t[:], eye_d[:])

        # ping-pong pe tiles in the NATIVE interleaved layout (even col 2k =
        # sin lane k, odd col 2k+1 = cos lane k) so merges read flat APs.
        # cos lanes k >= COSH hold constant 1.0 (memset once, never
        # overwritten: ScalarE only writes sin + cos-head lanes)
        pe_tiles = []
        for pp in range(3):
            pe_t = const_pool.tile([128, GB * E], BF16, tag=f"pe{pp}")
            nc.vector.memset(
                pe_t[:].rearrange("p (i e) -> p i e", i=GB)[
                    :, :, 2 * COSH + 1 : E : 2
                ],
                1.0,
            )
            pe_tiles.append(pe_t)

        # ---- software-pipelined emission over uneven stages (vt, b0, nb) ----
        # Stage k's production (angles, r4, sins, next-vtile diag builds) is
        # emitted BEFORE stage k-1's consumption (matmuls, merges, stores) so
        # no engine queue head-of-line blocks on a consumer stall.  The first
        # and last stages are half-size so the pipeline fill and drain ramps
        # are governed by a 4-item chain instead of an 8-item one.
        stages = [(0, 0, 4), (0, 4, 4), (0, 8, 8)]
        for vt in range(1, VT - 1):
            stages += [(vt, 0, 8), (vt, 8, 8)]
        stages += [(VT - 1, 0, 8), (VT - 1, 8, 4), (VT - 1, 12, 4)]
        tabs: dict = {}
        dmats: dict = {}
        staged: dict = {}

        def seq_view(vt):
            return seq_t[:, vt * B * 3 : (vt + 1) * B * 3].rearrange(
                "p (b c) -> p b c", c=3
            )

        def load_tables(vt):
            fw_t = tab_pool.tile([128, E], BF16, tag="fw")
            nc.sync.dma_start(fw_t[:], fw_d[vt * 128 : (vt + 1) * 128, :])
            tw_t = tab_pool.tile([128, E], BF16, tag="tw")
            nc.sync.dma_start(tw_t[:], tw_d[vt * 128 : (vt + 1) * 128, :])
            bs_t = tab_pool.tile([128, E], BF16, tag="bs")
            nc.sync.dma_start(bs_t[:], bs_d[vt * 128 : (vt + 1) * 128, :])
            cmb_t = tab_pool.tile([128, B * 2 * KLO], BF16, tag="cmb")
            nc.sync.dma_start(
                cmb_t[:], cmb_d[:, vt * B * 2 * KLO : (vt + 1) * B * 2 * KLO]
            )
            tabs[vt] = (fw_t, tw_t, bs_t, cmb_t)

        def alloc_dmats(vt):
            dmats[vt] = {}
            for ch in (0, 2):
                d_t = dmat_pool.tile([128, B * 128], BF16, tag=f"d{ch}")
                dmats[vt][ch] = d_t

        def build_d(vt):
            # diag build D[p, b*128+q] = eye[p,q] * s_ch[p,b]:
            # batched broadcast chunks on GPSIMD for every v-tile
            sv = seq_view(vt)
            # first-half chunks (batches 0..7) for BOTH channels first: that
            # is all the g=0 stage needs, so its matmuls start ~4us earlier
            for h0 in (0, B // 2):
                for ch in (0, 2):
                    d3 = dmats[vt][ch][:].rearrange("p (b q) -> p b q", b=B)
                    eye_b = eye_t[:].unsqueeze(1).broadcast_to((128, B // 2, 128))
                    s_b = sv[:, h0 : h0 + B // 2, ch : ch + 1].broadcast_to(
                        (128, B // 2, 128)
                    )
                    nc.gpsimd.tensor_tensor(
                        d3[:, h0 : h0 + B // 2, :], eye_b, s_b, Alu.mult
                    )

        def emit_stage(k, vt, b0, nb):
            ang_g = ang_pool.tile([128, GB * 2 * KLO], F32, tag="ang")
            r4_g = ang_pool.tile([128, GB * 2 * KLO], F32, tag="r4")
            cmb_t = tabs[vt][3]
            pe_g = pe_tiles[k % 3]
            pe3 = pe_g[:].rearrange("p (i e) -> p i e", i=GB)[:, 0:nb, :]
            sv = seq_view(vt)
            s1b = sv[:, b0 : b0 + nb, 1:2]  # [128, nb, 1] positional channel
            # staging op2': ang_k for k >= KLO straight into the pe sin
            # lanes (sin(x)=x holds for k >= KSIN; lanes [KLO,KSIN) are
            # Sin-ed in place below; lanes [KSIN,COSH) feed cos first).
            # Per-item on ScalarE: the DVE queue (merges) is the pacer.
            for i, b in enumerate(range(b0, b0 + nb)):
                col = vt * B * 3 + b * 3 + 1
                nc.scalar.mul(
                    pe_g[:, i * E + 2 * KLO : (i + 1) * E : 2],
                    dv_t[:, 0:KHI],
                    seq_t[:, col : col + 1],
                )
            # grouped staging op1': lo r0 lanes (dv2' scaled 2/pi) — on the
            # DVE: GPSIMD runs these ~3x slower and its SBUF-port traffic
            # stalls concurrent DVE work
            nc.vector.tensor_tensor(
                ang_g[:, 0 : nb * 2 * KLO].rearrange("p (i l) -> p i l", i=nb),
                dv_t[:, KHI:KX].unsqueeze(1).broadcast_to((128, nb, 2 * KLO)),
                s1b.broadcast_to((128, nb, 2 * KLO)),
                Alu.mult,
            )
            # lo codes r' = r0 + combo, one grouped add
            nc.gpsimd.tensor_tensor(
                r4_g[:, 0 : nb * 2 * KLO].rearrange("p (i l) -> p i l", i=nb),
                ang_g[:, 0 : nb * 2 * KLO].rearrange("p (i l) -> p i l", i=nb),
                cmb_t[
                    :, b0 * 2 * KLO : (b0 + nb) * 2 * KLO
                ].rearrange("p (i l) -> p i l", i=nb),
                Alu.add,
            )
            # cos head: reads the staged angles from the sin lanes (before
            # the in-place Sin consumes them), writes the odd cos columns
            nc.scalar.activation(
                pe3[:, :, 2 * KLO + 1 : 2 * COSH : 2],
                pe3[:, :, 2 * KLO : 2 * COSH : 2],
                mybir.ActivationFunctionType.Sin,
                bias=hpi_t[:],
                scale=1.0,
            )
            # sin head, in place over the staged angles
            nc.scalar.activation(
                pe3[:, :, 2 * KLO : 2 * KSIN : 2],
                pe3[:, :, 2 * KLO : 2 * KSIN : 2],
                mybir.ActivationFunctionType.Sin,
                bias=zero_t[:],
                scale=1.0,
            )
            # lo block: r4 layout per item is [48 sin | 48 cos]
            nc.scalar.activation(
                pe3[:, :, 0 : 2 * KLO].rearrange("p i (q h) -> p i h q", h=2),
                r4_g[:, 0 : nb * 2 * KLO].rearrange(
                    "p (i h q) -> p i h q", i=nb, h=2
                ),
                mybir.ActivationFunctionType.Sin,
                bias=zero_t[:],
                scale=HALF_PI * SIN_SAFETY,
            )
            return pe_g

        def emit_items(vt, b0, nb, pe_g):
            fw_t, tw_t, bs_t, _ = tabs[vt]
            # bf16 output written per merge-pair straight to item-keyed DRAM
            # blocks: uniform 393KB stores keep the tail drain overlapped
            o_g = out_pool.tile([128, GB * E], BF16, tag="o")
            for i, b in enumerate(range(b0, b0 + nb)):
                # items are merged in PAIRS from one 4-bank psum tile
                # (item0 at cols [0:768), item1 at [1024:1792)): one DVE
                # tensor_tensor per pair amortizes the 120-cycle PSUM-access
                # overhead and halves the DVE op count
                if i % 2 == 0:
                    psp = psum_pool.tile([128, 2048], F32, tag="ps")
                base = (i % 2) * 1024
                # psum += diag(s0)@fw + diag(s2)@tw + I@bsum, 512/256 column
                # splits per PSUM bank, stationary-major order
                for st_t, mov_t, first, last in (
                    (dmats[vt][0][:, b * 128 : (b + 1) * 128], fw_t, True, False),
                    (dmats[vt][2][:, b * 128 : (b + 1) * 128], tw_t, False, False),
                    (eye_t[:], bs_t, False, True),
                ):
                    for lo, hi in ((0, 512), (512, E)):
                        nc.tensor.matmul(
                            psp[:, base + lo : base + hi],
                            st_t,
                            mov_t[:, lo:hi],
                            start=first,
                            stop=last,
                        )
                if i % 2 == 1:
                    # merge both items of the pair (flat APs: pe is natively
                    # interleaved; psum f32 caps the TT at 1x)
                    nc.vector.tensor_tensor(
                        o_g[:, (i - 1) * E : (i + 1) * E].rearrange(
                            "p (j e) -> p j e", j=2
                        ),
                        psp[:].rearrange("p (j x) -> p j x", j=2)[:, :, 0:E],
                        pe_g[:, (i - 1) * E : (i + 1) * E].rearrange(
                            "p (j e) -> p j e", j=2
                        ),
                        Alu.add,
                    )
                if i % 2 == 1:
                    it = vt * B + b  # global item index of the pair's 2nd item
                    nc.sync.dma_start(
                        out_d[it - 1 : it + 1].rearrange("j p e -> p j e"),
                        o_g[:, (i - 1) * E : (i + 1) * E].rearrange(
                            "p (j e) -> p j e", j=2
                        ),
                    )

        load_tables(0)
        alloc_dmats(0)
        build_d(0)
        for k, (vt, b0, nb) in enumerate(stages):
            if b0 == 0 and vt + 1 < VT:
                load_tables(vt + 1)
            staged[(vt, b0)] = emit_stage(k, vt, b0, nb)
            if b0 == 0 and vt + 1 < VT:
                alloc_dmats(vt + 1)
                build_d(vt + 1)
            if k >= 1:
                pvt, pb0, pnb = stages[k - 1]
                emit_items(pvt, pb0, pnb, staged.pop((pvt, pb0)))
        pvt, pb0, pnb = stages[-1]
        emit_items(pvt, pb0, pnb, staged.pop((pvt, pb0)))


    nc.finalize()
    return nc


_NC_CACHE: list = []


def _get_nc():
    if not _NC_CACHE:
        _NC_CACHE.append(build_bass())
    return _NC_CACHE[0]


def make_in_maps(sequence, flux_w, flux_b, time_w, time_b):
    import ml_dtypes

    sequence = np.asarray(sequence, dtype=np.float32)
    flux_w = np.asarray(flux_w, dtype=np.float32)
    time_w = np.asarray(time_w, dtype=np.float32)
    bsum = np.asarray(flux_b, dtype=np.float32) + np.asarray(time_b, dtype=np.float32)

    s1_all = sequence[:, :, 1]
    assert np.abs(s1_all).max() < S1_LIMIT, (
        f"positional channel exceeds direct-Sin range: {np.abs(s1_all).max():.3f} "
        f">= {S1_LIMIT:.3f}; raise KLO"
    )

    div = np.exp(
        np.arange(0, E, 2, dtype=np.float32) * np.float32(-math.log(10000.0) / E)
    ).astype(np.float32)
    # dv_full = [ div[KLO:] (336) | 48 lo sin lanes * 2/pi | 48 lo cos * 2/pi ]
    dv2p = (np.concatenate([div[:KLO], div[:KLO]]) * np.float32(2.0 / math.pi)).astype(
        np.float32
    )
    dv_ext = np.concatenate([div[KLO:], dv2p]).astype(np.float32)
    dv_rep = np.ascontiguousarray(np.broadcast_to(dv_ext, (128, KHI + 2 * KLO)))
    eye_bf = np.eye(128, dtype=np.float32).astype(ml_dtypes.bfloat16)

    # combo_n[b,v,h*KLO+k] = j - 4*rint((s1*d_k + j*pi/2)/2pi), j = h (0=sin,1=cos)
    jj = np.concatenate([np.zeros(KLO, np.float64), np.ones(KLO, np.float64)])
    dd = np.concatenate([div[:KLO], div[:KLO]]).astype(np.float64)
    ang = s1_all[:, :, None].astype(np.float64) * dd[None, None, :] + jj * (
        math.pi / 2.0
    )
    n = np.rint(ang / TWO_PI)
    combo_n = (jj[None, None, :] - 4.0 * n).astype(np.float32)
    assert np.abs(combo_n).max() <= 16, "combo codes exceed bf16-exact range"
    combo_bf = combo_n.astype(ml_dtypes.bfloat16)  # small ints: bf16-exact

    fw_bf = flux_w.astype(ml_dtypes.bfloat16)
    tw_bf = time_w.astype(ml_dtypes.bfloat16)
    bs_bf = bsum.astype(ml_dtypes.bfloat16)

    in_maps = []
    for c in range(N_CORES):
        v0, v1 = c * V_SHARD, (c + 1) * V_SHARD
        # [B, 512, 3] -> [128p, vt*B*3 + b*3 + ch]
        s = sequence[:, v0:v1, :].reshape(B, VT, 128, 3)
        seq_r = np.ascontiguousarray(s.transpose(2, 1, 0, 3)).reshape(128, VT * B * 3)
        # combo [B, 512, 2*KLO] -> [128p, (vt*B + b)*2*KLO + lane]
        cmb = combo_bf[:, v0:v1, :].reshape(B, VT, 128, 2 * KLO)
        cmb_r = np.ascontiguousarray(cmb.transpose(2, 1, 0, 3)).reshape(
            128, VT * B * 2 * KLO
        )
        in_maps.append(
            {
                "seq": seq_r,
                "fw": np.ascontiguousarray(fw_bf[v0:v1]),
                "tw": np.ascontiguousarray(tw_bf[v0:v1]),
                "bs": np.ascontiguousarray(bs_bf[v0:v1]),
                "dv": dv_rep,
                "combo": cmb_r,
                "eye": eye_bf,
            }
        )
    return in_maps


def run(in_maps, trace: bool = False):
    nc = _get_nc()
    return run_bass_kernel_spmd(nc, in_maps, list(range(N_CORES)), trace=trace)


def assemble(res) -> np.ndarray:
    """Reassemble per-core [VT*B, 128, E] bf16 item blocks into [B, V, E] f32."""
    cores = []
    for c in range(N_CORES):
        arr = np.asarray(res.results[c]["out"]).astype(np.float32)
        # [vt*B+b, p, e] -> [b, v, e] with v = vt*128+p
        arr = arr.reshape(VT, B, 128, E).transpose(1, 0, 2, 3)
        cores.append(arr.reshape(B, V_SHARD, E))
    return np.ascontiguousarray(np.concatenate(cores, axis=1))


def kernel(sequence, flux_w, flux_b, time_w, time_b) -> np.ndarray:
    in_maps = make_in_maps(sequence, flux_w, flux_b, time_w, time_b)
    res = run(in_maps)
    return assemble(res)

